# revision 5
# baseline (speedup 1.0000x reference)
"""CVAE GNN (2x GraphConvolution encoder + reparam + decoder MLP) on 8 trn2
NeuronCores via Bass/Tile.

Strategy:
  - Shard nodes by receiver range (12500/core), re-ordered per core by
    in-degree (descending) so that 128-node tiles have near-uniform degree.
  - Dense layers are computed on each core for its own shard; the 20-dim
    hidden table is AllGathered so every core holds the full table in DRAM.
  - Edge aggregation (segment_sum of h[senders] over receivers) runs as
    one indirect-DMA gather per (tile, slot): each instruction gathers 128
    rows (one per partition = one per receiver node of the tile), and a
    strided tensor_reduce sums the K_t slots per node.  Padding slots point
    at zero rows appended to each shard of the table.
  - Degree normalization factors (pure functions of the index arrays) are
    computed on the host and staged as per-node scalars.

The walrus build in this container rejects instructions with more than one
semaphore wait; tile_patch (inlined below) splits them into discrete
wait_ge instructions.
"""
import os

import numpy as np

import concourse.bass as bass
import concourse.tile as tile
from concourse import mybir, bass_utils
from concourse.masks import make_identity

# ---------------------------------------------------------------------------
# walrus single-wait workaround (inlined so kernel.py is self-contained)
# ---------------------------------------------------------------------------


def _split_waits(tc, inst, keep: int):
    si = getattr(inst, "sync_info", None)
    if si is None:
        return
    waits = list(si.on_wait)
    if len(waits) <= keep:
        return
    splittable = [w for w in waits if w.wait_mode == "sem-ge-imm" and w.wait_reg is None]
    unsplittable = [w for w in waits if not (w.wait_mode == "sem-ge-imm" and w.wait_reg is None)]
    n_keep = max(0, keep - len(unsplittable))
    emit = splittable[:len(splittable) - n_keep]
    keep_on_inst = splittable[len(splittable) - n_keep:]
    if not emit:
        return
    name2sem = {s.name: s for s in tc.sems.allocated().values()}
    eng = tc.nc.engines[inst.engine]
    for w in emit:
        sem = name2sem.get(w.ant_name)
        if sem is None:
            keep_on_inst.append(w)
            continue
        eng.wait_ge(sem, w.wait_value)
    si.on_wait = unsplittable + keep_on_inst


_orig_commit_and_lower = tile.TileContext._commit_and_lower


def _patched_commit_and_lower(self, inst, bb, *args, **kwargs):
    _split_waits(self, inst, keep=1)
    return _orig_commit_and_lower(self, inst, bb, *args, **kwargs)


def _patched_drain_and_barrier(self, tick_clock, wait_clock):
    from concourse.tile import ScopedClock
    drain_inst = self.nc.sync.drain()
    wait_clock.add_sem_waits(drain_inst.ins, ScopedClock({None: tick_clock.global_clock}))
    si = drain_inst.ins.sync_info
    waits = list(si.on_wait)
    si.on_wait = []
    name2sem = {s.name: s for s in self.sems.allocated().values()}
    for w in waits:
        self.nc.sync.wait_ge(name2sem[w.ant_name], w.wait_value)
    self.nc.all_engine_barrier()
    popped = self.nc._tile_sem_poison_stack.pop()
    assert popped is self._sem_poison
    self.nc.clear_and_free_semaphores(list(self.sems.allocated().values()))
    self.nc.all_engine_barrier()


tile.TileContext._commit_and_lower = _patched_commit_and_lower
tile.TileContext._drain_and_barrier = _patched_drain_and_barrier

# ---------------------------------------------------------------------------
# problem constants
# ---------------------------------------------------------------------------
NC = 8
N_NODES = 100000
D_FEAT = 256
GC_HID = 20
Z_DIM = 32
DEC_HID = 40
EXPR_DIM = 256
SIGMA_BOUND = 0.0001

P = 128
SH_REAL = N_NODES // NC            # 12500 real nodes per core
TILES = (SH_REAL + P - 1) // P     # 98
SH_PAD = TILES * P                 # 12544 (rows incl. dummy pad nodes)
ZPAD = 32                          # zero rows appended per shard (pad targets)
SHP = SH_PAD + ZPAD                # shard rows in the gathered table
TBL = NC * SHP                     # full table rows
ZROW_G = SH_PAD                    # global id of a guaranteed-zero row (core 0)

F32 = mybir.dt.float32
I32 = mybir.dt.int32


# ---------------------------------------------------------------------------
# host-side graph preprocessing
# ---------------------------------------------------------------------------
def _preprocess(senders, receivers):
    """Build per-core permutations and gather-index matrices.

    Returns dict with per-core arrays and the common per-tile slot counts.
    """
    deg_s = np.bincount(senders, minlength=N_NODES)
    deg_r = np.bincount(receivers, minlength=N_NODES)
    rs = (1.0 / np.sqrt(np.maximum(deg_s, 1.0))).astype(np.float32)
    rr = (1.0 / np.sqrt(np.maximum(deg_r, 1.0))).astype(np.float32)

    owner = receivers // SH_REAL

    # per-core node ordering by descending in-degree
    ids = []          # per core: global node id at each rank (length SH_REAL)
    rank_g = np.empty(N_NODES, np.int64)  # global node -> rank within its core
    for c in range(NC):
        lo = c * SH_REAL
        local_deg = deg_r[lo:lo + SH_REAL]
        order = np.argsort(-local_deg, kind="stable")
        ids.append(lo + order)
        rank_g[lo + order] = np.arange(SH_REAL)

    g_of = (np.int64(SHP) * (np.arange(N_NODES) // SH_REAL) + rank_g).astype(np.int64)

    # per-core per-tile max degree, then common max across cores
    K_t = np.zeros(TILES, np.int64)
    percore = []
    for c in range(NC):
        m = owner == c
        s_c = senders[m]
        r_c = receivers[m]
        rk = rank_g[r_c]                       # receiver rank within core
        order = np.argsort(rk, kind="stable")
        s_sorted = s_c[order]
        rk_sorted = rk[order]
        counts = np.bincount(rk_sorted, minlength=SH_PAD)
        starts = np.concatenate([[0], np.cumsum(counts)[:-1]])
        j = np.arange(len(rk_sorted)) - starts[rk_sorted]
        percore.append((s_sorted, rk_sorted, j, counts))
        tile_max = counts.reshape(TILES, P).max(axis=1)
        K_t = np.maximum(K_t, tile_max)

    col_off = np.concatenate([[0], np.cumsum(K_t)[:-1]])
    NI = int(K_t.sum())

    idx_mats = []
    for c in range(NC):
        s_sorted, rk_sorted, j, _counts = percore[c]
        A = np.full((P, NI), ZROW_G, np.int32)       # [partition, instr]
        t = rk_sorted // P
        part = rk_sorted % P
        col = col_off[t] + j
        A[part, col] = g_of[s_sorted].astype(np.int32)
        idx_mats.append(A)

    return {
        "ids": ids, "rs": rs, "rr": rr,
        "K_t": K_t.astype(int), "NI": NI, "idx": idx_mats,
    }


# ---------------------------------------------------------------------------
# device program
# ---------------------------------------------------------------------------
_PROGRAM_CACHE = {}


def _build_program(K_t, NI):
    key = (tuple(K_t), NI)
    if key in _PROGRAM_CACHE:
        return _PROGRAM_CACHE[key]

    try:
        import jax
        jax.config.update("jax_compilation_cache_dir", "/tmp/jax_neff_cache")
        jax.config.update("jax_persistent_cache_min_compile_time_secs", 1.0)
        jax.config.update("jax_persistent_cache_min_entry_size_bytes", 0)
    except Exception:
        pass

    nc = bass.Bass("TRN2", num_devices=NC)

    x_in = nc.declare_dram_parameter("x", [SH_PAD, D_FEAT], F32, isOutput=False)
    eps_in = nc.declare_dram_parameter("eps", [SH_PAD, Z_DIM], F32, isOutput=False)
    rs_in = nc.declare_dram_parameter("rs", [SH_PAD, 1], F32, isOutput=False)
    rr_in = nc.declare_dram_parameter("rr", [SH_PAD, 1], F32, isOutput=False)
    idx_in = nc.declare_dram_parameter("idx", [P, NI], I32, isOutput=False)
    w1_in = nc.declare_dram_parameter("w1", [D_FEAT, GC_HID], F32, isOutput=False)
    b1_in = nc.declare_dram_parameter("b1", [1, GC_HID], F32, isOutput=False)
    w2_in = nc.declare_dram_parameter("w2", [GC_HID, GC_HID], F32, isOutput=False)
    b2_in = nc.declare_dram_parameter("b2", [1, GC_HID], F32, isOutput=False)
    # mu/ls combined: x-part [256, 64], h-part [20, 64], bias [1, 64]
    wx_in = nc.declare_dram_parameter("wx", [D_FEAT, 2 * Z_DIM], F32, isOutput=False)
    wh_in = nc.declare_dram_parameter("wh", [GC_HID, 2 * Z_DIM], F32, isOutput=False)
    bml_in = nc.declare_dram_parameter("bml", [1, 2 * Z_DIM], F32, isOutput=False)
    wd1_in = nc.declare_dram_parameter("wd1", [Z_DIM, DEC_HID], F32, isOutput=False)
    bd1_in = nc.declare_dram_parameter("bd1", [1, DEC_HID], F32, isOutput=False)
    wd2_in = nc.declare_dram_parameter("wd2", [DEC_HID, EXPR_DIM], F32, isOutput=False)
    bd2_in = nc.declare_dram_parameter("bd2", [1, EXPR_DIM], F32, isOutput=False)

    xs_out = nc.declare_dram_parameter("xs", [SH_PAD, EXPR_DIM], F32, isOutput=True)
    mu_out = nc.declare_dram_parameter("mu", [SH_PAD, Z_DIM], F32, isOutput=True)
    ls_out = nc.declare_dram_parameter("ls", [SH_PAD, Z_DIM], F32, isOutput=True)

    # collective bounce buffers
    h0_sh = nc.dram_tensor("h0_sh", [SHP, GC_HID], F32)
    H0 = nc.dram_tensor("H0", [TBL, GC_HID], F32)
    h1_sh = nc.dram_tensor("h1_sh", [SHP, GC_HID], F32)
    H1 = nc.dram_tensor("H1", [TBL, GC_HID], F32)

    K_max = int(max(K_t))
    col_off = np.concatenate([[0], np.cumsum(K_t)[:-1]]).astype(int)

    def bcast(dram_ap, n):
        return bass.AP(tensor=dram_ap.tensor, offset=dram_ap.offset,
                       ap=[[0, P]] + list(dram_ap.ap[1:]))

    from contextlib import ExitStack
    trace_sim = bool(os.environ.get("KERNEL_TRACE_SIM"))
    with tile.TileContext(nc, trace_sim=trace_sim) as tc, ExitStack() as stack:
        const = stack.enter_context(tc.tile_pool(name="const", bufs=1))
        io = stack.enter_context(tc.tile_pool(name="io", bufs=3))
        work = stack.enter_context(tc.tile_pool(name="work", bufs=3))
        wide_p = stack.enter_context(tc.tile_pool(name="wide", bufs=2))
        keep = stack.enter_context(tc.tile_pool(name="keep", bufs=1))
        ps = stack.enter_context(tc.tile_pool(name="ps", bufs=3, space="PSUM"))
        pst = stack.enter_context(tc.tile_pool(name="pst", bufs=2, space="PSUM"))

        ident = const.tile([P, P], F32)
        make_identity(nc, ident[:])

        # weights
        w1 = const.tile([P, 2, GC_HID], F32)
        nc.sync.dma_start(out=w1[:], in_=w1_in[:, :].rearrange("(a k) n -> k a n", k=P))
        wx = const.tile([P, 2, 2 * Z_DIM], F32)
        nc.sync.dma_start(out=wx[:], in_=wx_in[:, :].rearrange("(a k) n -> k a n", k=P))
        w2 = const.tile([GC_HID, GC_HID], F32)
        nc.sync.dma_start(out=w2[:], in_=w2_in[:, :])
        wh = const.tile([GC_HID, 2 * Z_DIM], F32)
        nc.sync.dma_start(out=wh[:], in_=wh_in[:, :])
        wd1 = const.tile([Z_DIM, DEC_HID], F32)
        nc.sync.dma_start(out=wd1[:], in_=wd1_in[:, :])
        wd2 = const.tile([DEC_HID, EXPR_DIM], F32)
        nc.sync.dma_start(out=wd2[:], in_=wd2_in[:, :])
        # replicated biases
        b1r = const.tile([P, GC_HID], F32)
        nc.gpsimd.dma_start(out=b1r[:], in_=bcast(b1_in[:, :], GC_HID))
        b2r = const.tile([P, GC_HID], F32)
        nc.gpsimd.dma_start(out=b2r[:], in_=bcast(b2_in[:, :], GC_HID))
        bmlr = const.tile([P, 2 * Z_DIM], F32)
        nc.gpsimd.dma_start(out=bmlr[:], in_=bcast(bml_in[:, :], 2 * Z_DIM))
        bd1r = const.tile([P, DEC_HID], F32)
        nc.gpsimd.dma_start(out=bd1r[:], in_=bcast(bd1_in[:, :], DEC_HID))
        bd2r = const.tile([P, EXPR_DIM], F32)
        nc.gpsimd.dma_start(out=bd2r[:], in_=bcast(bd2_in[:, :], EXPR_DIM))
        # per-node scalars
        rs_all = const.tile([P, TILES], F32)
        nc.sync.dma_start(out=rs_all[:], in_=rs_in[:, 0].rearrange("(t p) -> p t", p=P))
        rr_all = const.tile([P, TILES], F32)
        nc.sync.dma_start(out=rr_all[:], in_=rr_in[:, 0].rearrange("(t p) -> p t", p=P))
        # gather indices
        idxt = const.tile([P, NI], I32)
        nc.sync.dma_start(out=idxt[:], in_=idx_in[:, :])
        # persistent x-part of mu/ls
        mlx = keep.tile([P, TILES, 2 * Z_DIM], F32)

        zeros20 = const.tile([ZPAD, GC_HID], F32)
        nc.vector.memset(zeros20[:], 0.0)

        def softmax_relu_scale(hp, bias_rep, scale_col, out_tile):
            """out = softmax(relu(hp + bias), axis=-1) * scale_col.

            hp: PSUM [P, GC_HID]; bias_rep: [P, GC_HID]; scale_col: [P, 1].
            """
            hs = work.tile([P, GC_HID], F32, tag="hs")
            nc.vector.tensor_tensor(out=hs[:], in0=hp[:], in1=bias_rep[:],
                                    op=mybir.AluOpType.add)
            nc.vector.tensor_scalar_max(hs[:], hs[:], 0.0)
            mx = work.tile([P, 1], F32, tag="mx")
            nc.vector.tensor_reduce(out=mx[:], in_=hs[:], axis=mybir.AxisListType.X,
                                    op=mybir.AluOpType.max)
            negmx = work.tile([P, 1], F32, tag="negmx")
            nc.vector.tensor_scalar_mul(negmx[:], mx[:], -1.0)
            ex = work.tile([P, GC_HID], F32, tag="ex")
            nc.scalar.activation(out=ex[:], in_=hs[:],
                                 func=mybir.ActivationFunctionType.Exp,
                                 bias=negmx[:, 0:1], scale=1.0)
            sm = work.tile([P, 1], F32, tag="sm")
            nc.vector.tensor_reduce(out=sm[:], in_=ex[:], axis=mybir.AxisListType.X,
                                    op=mybir.AluOpType.add)
            rcp = work.tile([P, 1], F32, tag="rcp")
            nc.vector.reciprocal(rcp[:], sm[:])
            nc.vector.tensor_scalar(out=out_tile[:], in0=ex[:], scalar1=rcp[:, 0:1],
                                    scalar2=scale_col, op0=mybir.AluOpType.mult,
                                    op1=mybir.AluOpType.mult)

        KW = min(K_max, 256)

        def aggregate(t, table, out_tile):
            """out_tile[p,:] = sum over slots of table[idx] ; scaled by rr."""
            kt = int(K_t[t])
            if kt == 0:
                nc.vector.memset(out_tile[:], 0.0)
                return
            for c0 in range(0, kt, KW):
                cn = min(KW, kt - c0)
                wide = wide_p.tile([P, KW, GC_HID], F32, tag="wide")
                for j in range(cn):
                    col = col_off[t] + c0 + j
                    nc.gpsimd.indirect_dma_start(
                        out=wide[:, j, :], out_offset=None, in_=table[:, :],
                        in_offset=bass.IndirectOffsetOnAxis(
                            ap=idxt[:, col:col + 1], axis=0))
                if c0 == 0:
                    nc.vector.tensor_reduce(
                        out=out_tile[:], in_=wide[:, :cn, :].rearrange("p k f -> p f k"),
                        axis=mybir.AxisListType.X, op=mybir.AluOpType.add)
                else:
                    part = work.tile([P, GC_HID], F32, tag="aggpart")
                    nc.vector.tensor_reduce(
                        out=part[:], in_=wide[:, :cn, :].rearrange("p k f -> p f k"),
                        axis=mybir.AxisListType.X, op=mybir.AluOpType.add)
                    nc.vector.tensor_tensor(out=out_tile[:], in0=out_tile[:],
                                            in1=part[:], op=mybir.AluOpType.add)
            nc.vector.tensor_scalar_mul(out_tile[:], out_tile[:], rr_all[:, t:t + 1])

        def pe_transpose(src_ap, rows, cols, tag):
            """[rows<=128, cols<=128] SBUF -> [cols, rows] SBUF via PE."""
            pt = pst.tile([cols, P], F32, space="PSUM", tag="ptrans")
            nc.tensor.transpose(out=pt[:, :rows], in_=src_ap, identity=ident[:, :rows])
            st = work.tile([cols, P], F32, tag=tag)
            nc.vector.tensor_copy(out=st[:, :rows], in_=pt[:, :rows])
            return st

        # ---------------- phase 1: h0 = smax(relu(xW1+b1))*rs ; mlx = x@Wx ---
        for t in range(TILES):
            xt = io.tile([P, D_FEAT], F32, tag="xt")
            nc.sync.dma_start(out=xt[:], in_=x_in[t * P:(t + 1) * P, :])
            xT = work.tile([P, 2, P], F32, tag="xT")
            for a in range(2):
                pt = pst.tile([P, P], F32, space="PSUM", tag="ptrans")
                nc.tensor.transpose(out=pt[:], in_=xt[:, a * P:(a + 1) * P],
                                    identity=ident[:])
                nc.vector.tensor_copy(out=xT[:, a, :], in_=pt[:])
            h0p = ps.tile([P, GC_HID], F32, space="PSUM", tag="mm")
            for a in range(2):
                nc.tensor.matmul(out=h0p[:], lhsT=xT[:, a, :], rhs=w1[:, a, :],
                                 start=(a == 0), stop=(a == 1))
            mlp = ps.tile([P, 2 * Z_DIM], F32, space="PSUM", tag="mm")
            for a in range(2):
                nc.tensor.matmul(out=mlp[:], lhsT=xT[:, a, :], rhs=wx[:, a, :],
                                 start=(a == 0), stop=(a == 1))
            nc.vector.tensor_copy(out=mlx[:, t, :], in_=mlp[:])
            h0s = work.tile([P, GC_HID], F32, tag="hout")
            softmax_relu_scale(h0p, b1r, rs_all[:, t:t + 1], h0s)
            nc.sync.dma_start(out=h0_sh[t * P:(t + 1) * P, :], in_=h0s[:])
        nc.sync.dma_start(out=h0_sh[SH_PAD:SHP, :], in_=zeros20[:])

        # ---------------- phase 2: allgather --------------------------------
        nc.gpsimd.collective_compute(
            "AllGather", mybir.AluOpType.bypass,
            replica_groups=[list(range(NC))],
            ins=[h0_sh[:, :]], outs=[H0[:, :]],
        )

        # ---------------- phase 3: aggregate; gc2; publish h1 ----------------
        for t in range(TILES):
            m1 = work.tile([P, GC_HID], F32, tag="m1")
            aggregate(t, H0, m1)
            m1T = pe_transpose(m1[:], P, GC_HID, tag="m1T")
            h1p = ps.tile([P, GC_HID], F32, space="PSUM", tag="mm")
            nc.tensor.matmul(out=h1p[:], lhsT=m1T[:, :], rhs=w2[:, :],
                             start=True, stop=True)
            h1s = work.tile([P, GC_HID], F32, tag="hout")
            softmax_relu_scale(h1p, b2r, rs_all[:, t:t + 1], h1s)
            nc.sync.dma_start(out=h1_sh[t * P:(t + 1) * P, :], in_=h1s[:])
        nc.sync.dma_start(out=h1_sh[SH_PAD:SHP, :], in_=zeros20[:])

        # ---------------- phase 4: allgather --------------------------------
        nc.gpsimd.collective_compute(
            "AllGather", mybir.AluOpType.bypass,
            replica_groups=[list(range(NC))],
            ins=[h1_sh[:, :]], outs=[H1[:, :]],
        )

        # ---------------- phase 5+6: aggregate; heads ------------------------
        for t in range(TILES):
            h2 = work.tile([P, GC_HID], F32, tag="m1")
            aggregate(t, H1, h2)
            h2T = pe_transpose(h2[:], P, GC_HID, tag="m1T")
            mlp2 = ps.tile([P, 2 * Z_DIM], F32, space="PSUM", tag="mm")
            nc.tensor.matmul(out=mlp2[:], lhsT=h2T[:, :], rhs=wh[:, :],
                             start=True, stop=True)
            muls = work.tile([P, 2 * Z_DIM], F32, tag="muls")
            nc.vector.tensor_tensor(out=muls[:], in0=mlp2[:], in1=mlx[:, t, :],
                                    op=mybir.AluOpType.add)
            nc.vector.tensor_tensor(out=muls[:], in0=muls[:], in1=bmlr[:],
                                    op=mybir.AluOpType.add)
            nc.sync.dma_start(out=mu_out[t * P:(t + 1) * P, :], in_=muls[:, :Z_DIM])
            nc.sync.dma_start(out=ls_out[t * P:(t + 1) * P, :], in_=muls[:, Z_DIM:])
            # z = mu + (SIGMA_BOUND + exp(0.5 ls)) * eps
            sig = work.tile([P, Z_DIM], F32, tag="sig")
            nc.scalar.activation(out=sig[:], in_=muls[:, Z_DIM:],
                                 func=mybir.ActivationFunctionType.Exp,
                                 bias=0.0, scale=0.5)
            nc.vector.tensor_scalar_add(sig[:], sig[:], SIGMA_BOUND)
            epst = io.tile([P, Z_DIM], F32, tag="epst")
            nc.sync.dma_start(out=epst[:], in_=eps_in[t * P:(t + 1) * P, :])
            z = work.tile([P, Z_DIM], F32, tag="z")
            nc.vector.tensor_tensor(out=z[:], in0=sig[:], in1=epst[:],
                                    op=mybir.AluOpType.mult)
            nc.vector.tensor_tensor(out=z[:], in0=z[:], in1=muls[:, :Z_DIM],
                                    op=mybir.AluOpType.add)
            zT = pe_transpose(z[:], P, Z_DIM, tag="zT")
            dp = ps.tile([P, DEC_HID], F32, space="PSUM", tag="mm")
            nc.tensor.matmul(out=dp[:], lhsT=zT[:, :], rhs=wd1[:, :],
                             start=True, stop=True)
            d = work.tile([P, DEC_HID], F32, tag="d")
            nc.vector.tensor_tensor(out=d[:], in0=dp[:], in1=bd1r[:],
                                    op=mybir.AluOpType.add)
            nc.vector.tensor_scalar_max(d[:], d[:], 0.0)
            dT = pe_transpose(d[:], P, DEC_HID, tag="dT")
            xp = ps.tile([P, EXPR_DIM], F32, space="PSUM", tag="mm")
            nc.tensor.matmul(out=xp[:], lhsT=dT[:, :], rhs=wd2[:, :],
                             start=True, stop=True)
            xo = work.tile([P, EXPR_DIM], F32, tag="xo")
            nc.vector.tensor_tensor(out=xo[:], in0=xp[:], in1=bd2r[:],
                                    op=mybir.AluOpType.add)
            nc.sync.dma_start(out=xs_out[t * P:(t + 1) * P, :], in_=xo[:])

    nc.finalize()
    _PROGRAM_CACHE[key] = nc
    return nc


# ---------------------------------------------------------------------------
# entry point
# ---------------------------------------------------------------------------
def kernel(nodes, senders, receivers, eps,
           W_gc1, b_gc1, W_gc2, b_gc2,
           W_mu, b_mu, W_ls, b_ls,
           W_d1, b_d1, W_d2, b_d2):
    nodes = np.asarray(nodes, np.float32)
    senders = np.asarray(senders, np.int64)
    receivers = np.asarray(receivers, np.int64)
    eps = np.asarray(eps, np.float32)

    pre = _preprocess(senders, receivers)
    nc = _build_program(pre["K_t"], pre["NI"])

    # split W_mu/W_ls into h-part (first GC_HID rows) and x-part
    W_mu = np.asarray(W_mu, np.float32)
    W_ls = np.asarray(W_ls, np.float32)
    wh = np.concatenate([W_mu[:GC_HID], W_ls[:GC_HID]], axis=1)      # [20, 64]
    wx = np.concatenate([W_mu[GC_HID:], W_ls[GC_HID:]], axis=1)      # [256, 64]
    bml = np.concatenate([np.asarray(b_mu, np.float32),
                          np.asarray(b_ls, np.float32)])[None, :]    # [1, 64]

    in_maps = []
    for c in range(NC):
        ids = pre["ids"][c]
        x_c = np.zeros((SH_PAD, D_FEAT), np.float32)
        x_c[:SH_REAL] = nodes[ids]
        eps_c = np.zeros((SH_PAD, Z_DIM), np.float32)
        eps_c[:SH_REAL] = eps[ids]
        rs_c = np.zeros((SH_PAD, 1), np.float32)
        rs_c[:SH_REAL, 0] = pre["rs"][ids]
        rr_c = np.zeros((SH_PAD, 1), np.float32)
        rr_c[:SH_REAL, 0] = pre["rr"][ids]
        in_maps.append({
            "x": x_c, "eps": eps_c, "rs": rs_c, "rr": rr_c,
            "idx": pre["idx"][c],
            "w1": np.asarray(W_gc1, np.float32),
            "b1": np.asarray(b_gc1, np.float32)[None, :],
            "w2": np.asarray(W_gc2, np.float32),
            "b2": np.asarray(b_gc2, np.float32)[None, :],
            "wx": wx, "wh": wh, "bml": bml,
            "wd1": np.asarray(W_d1, np.float32),
            "bd1": np.asarray(b_d1, np.float32)[None, :],
            "wd2": np.asarray(W_d2, np.float32),
            "bd2": np.asarray(b_d2, np.float32)[None, :],
        })

    res = bass_utils.run_bass_kernel_spmd(nc, in_maps, core_ids=list(range(NC)))

    Xs = np.empty((N_NODES, EXPR_DIM), np.float32)
    mu = np.empty((N_NODES, Z_DIM), np.float32)
    ls = np.empty((N_NODES, Z_DIM), np.float32)
    for c in range(NC):
        ids = pre["ids"][c]
        r = res.results[c]
        Xs[ids] = r["xs"][:SH_REAL]
        mu[ids] = r["mu"][:SH_REAL]
        ls[ids] = r["ls"][:SH_REAL]
    return Xs, mu, ls


# revision 6
# speedup vs baseline: 1.0029x; 1.0029x over previous
"""CVAE GNN (2x GraphConvolution encoder + reparam + decoder MLP) on 8 trn2
NeuronCores via Bass/Tile.

Strategy:
  - Shard nodes by receiver range (12500/core), re-ordered per core by
    in-degree (descending) so that 128-node tiles have near-uniform degree.
  - Dense layers are computed on each core for its own shard; the 20-dim
    hidden table is AllGathered so every core holds the full table in DRAM.
  - Edge aggregation (segment_sum of h[senders] over receivers) runs as
    one indirect-DMA gather per (tile, slot): each instruction gathers 128
    rows (one per partition = one per receiver node of the tile), and a
    strided tensor_reduce sums the K_t slots per node.  Padding slots point
    at zero rows appended to each shard of the table.
  - Degree normalization factors (pure functions of the index arrays) are
    computed on the host and staged as per-node scalars.

The walrus build in this container rejects instructions with more than one
semaphore wait; tile_patch (inlined below) splits them into discrete
wait_ge instructions.
"""
import os

import numpy as np

import concourse.bass as bass
import concourse.tile as tile
from concourse import mybir, bass_utils
from concourse.masks import make_identity

# ---------------------------------------------------------------------------
# walrus single-wait workaround (inlined so kernel.py is self-contained)
# ---------------------------------------------------------------------------


def _split_waits(tc, inst, keep: int):
    si = getattr(inst, "sync_info", None)
    if si is None:
        return
    waits = list(si.on_wait)
    if len(waits) <= keep:
        return
    splittable = [w for w in waits if w.wait_mode == "sem-ge-imm" and w.wait_reg is None]
    unsplittable = [w for w in waits if not (w.wait_mode == "sem-ge-imm" and w.wait_reg is None)]
    n_keep = max(0, keep - len(unsplittable))
    emit = splittable[:len(splittable) - n_keep]
    keep_on_inst = splittable[len(splittable) - n_keep:]
    if not emit:
        return
    name2sem = {s.name: s for s in tc.sems.allocated().values()}
    eng = tc.nc.engines[inst.engine]
    for w in emit:
        sem = name2sem.get(w.ant_name)
        if sem is None:
            keep_on_inst.append(w)
            continue
        eng.wait_ge(sem, w.wait_value)
    si.on_wait = unsplittable + keep_on_inst


_orig_commit_and_lower = tile.TileContext._commit_and_lower


def _patched_commit_and_lower(self, inst, bb, *args, **kwargs):
    _split_waits(self, inst, keep=1)
    return _orig_commit_and_lower(self, inst, bb, *args, **kwargs)


def _patched_drain_and_barrier(self, tick_clock, wait_clock):
    from concourse.tile import ScopedClock
    drain_inst = self.nc.sync.drain()
    wait_clock.add_sem_waits(drain_inst.ins, ScopedClock({None: tick_clock.global_clock}))
    si = drain_inst.ins.sync_info
    waits = list(si.on_wait)
    si.on_wait = []
    name2sem = {s.name: s for s in self.sems.allocated().values()}
    for w in waits:
        self.nc.sync.wait_ge(name2sem[w.ant_name], w.wait_value)
    self.nc.all_engine_barrier()
    popped = self.nc._tile_sem_poison_stack.pop()
    assert popped is self._sem_poison
    self.nc.clear_and_free_semaphores(list(self.sems.allocated().values()))
    self.nc.all_engine_barrier()


tile.TileContext._commit_and_lower = _patched_commit_and_lower
tile.TileContext._drain_and_barrier = _patched_drain_and_barrier

# ---------------------------------------------------------------------------
# problem constants
# ---------------------------------------------------------------------------
NC = 8
N_NODES = 100000
D_FEAT = 256
GC_HID = 20
Z_DIM = 32
DEC_HID = 40
EXPR_DIM = 256
SIGMA_BOUND = 0.0001

P = 128
SH_REAL = N_NODES // NC            # 12500 real nodes per core
TILES = (SH_REAL + P - 1) // P     # 98
SH_PAD = TILES * P                 # 12544 (rows incl. dummy pad nodes)
ZPAD = 32                          # zero rows appended per shard (pad targets)
SHP = SH_PAD + ZPAD                # shard rows in the gathered table
TBL = NC * SHP                     # full table rows
ZROW_G = SH_PAD                    # global id of a guaranteed-zero row (core 0)

F32 = mybir.dt.float32
I32 = mybir.dt.int32


# ---------------------------------------------------------------------------
# host-side graph preprocessing
# ---------------------------------------------------------------------------
def _preprocess(senders, receivers):
    """Build per-core permutations and gather-index matrices.

    Returns dict with per-core arrays and the common per-tile slot counts.
    """
    deg_s = np.bincount(senders, minlength=N_NODES)
    deg_r = np.bincount(receivers, minlength=N_NODES)
    rs = (1.0 / np.sqrt(np.maximum(deg_s, 1.0))).astype(np.float32)
    rr = (1.0 / np.sqrt(np.maximum(deg_r, 1.0))).astype(np.float32)

    owner = receivers // SH_REAL

    # per-core node ordering by descending in-degree
    ids = []          # per core: global node id at each rank (length SH_REAL)
    rank_g = np.empty(N_NODES, np.int64)  # global node -> rank within its core
    for c in range(NC):
        lo = c * SH_REAL
        local_deg = deg_r[lo:lo + SH_REAL]
        order = np.argsort(-local_deg, kind="stable")
        ids.append(lo + order)
        rank_g[lo + order] = np.arange(SH_REAL)

    g_of = (np.int64(SHP) * (np.arange(N_NODES) // SH_REAL) + rank_g).astype(np.int64)

    # per-core per-tile max degree, then common max across cores
    K_t = np.zeros(TILES, np.int64)
    percore = []
    for c in range(NC):
        m = owner == c
        s_c = senders[m]
        r_c = receivers[m]
        rk = rank_g[r_c]                       # receiver rank within core
        order = np.argsort(rk, kind="stable")
        s_sorted = s_c[order]
        rk_sorted = rk[order]
        counts = np.bincount(rk_sorted, minlength=SH_PAD)
        starts = np.concatenate([[0], np.cumsum(counts)[:-1]])
        j = np.arange(len(rk_sorted)) - starts[rk_sorted]
        percore.append((s_sorted, rk_sorted, j, counts))
        tile_max = counts.reshape(TILES, P).max(axis=1)
        K_t = np.maximum(K_t, tile_max)

    col_off = np.concatenate([[0], np.cumsum(K_t)[:-1]])
    NI = int(K_t.sum())

    idx_mats = []
    for c in range(NC):
        s_sorted, rk_sorted, j, _counts = percore[c]
        A = np.full((P, NI), ZROW_G, np.int32)       # [partition, instr]
        t = rk_sorted // P
        part = rk_sorted % P
        col = col_off[t] + j
        A[part, col] = g_of[s_sorted].astype(np.int32)
        idx_mats.append(A)

    return {
        "ids": ids, "rs": rs, "rr": rr,
        "K_t": K_t.astype(int), "NI": NI, "idx": idx_mats,
    }


# ---------------------------------------------------------------------------
# device program
# ---------------------------------------------------------------------------
_PROGRAM_CACHE = {}


def _build_program(K_t, NI):
    key = (tuple(K_t), NI)
    if key in _PROGRAM_CACHE:
        return _PROGRAM_CACHE[key]

    try:
        import jax
        jax.config.update("jax_compilation_cache_dir", "/tmp/jax_neff_cache")
        jax.config.update("jax_persistent_cache_min_compile_time_secs", 1.0)
        jax.config.update("jax_persistent_cache_min_entry_size_bytes", 0)
    except Exception:
        pass

    nc = bass.Bass("TRN2", num_devices=NC)

    x_in = nc.declare_dram_parameter("x", [SH_PAD, D_FEAT], F32, isOutput=False)
    eps_in = nc.declare_dram_parameter("eps", [SH_PAD, Z_DIM], F32, isOutput=False)
    rs_in = nc.declare_dram_parameter("rs", [SH_PAD, 1], F32, isOutput=False)
    rr_in = nc.declare_dram_parameter("rr", [SH_PAD, 1], F32, isOutput=False)
    idx_in = nc.declare_dram_parameter("idx", [P, NI], I32, isOutput=False)
    w1_in = nc.declare_dram_parameter("w1", [D_FEAT, GC_HID], F32, isOutput=False)
    b1_in = nc.declare_dram_parameter("b1", [1, GC_HID], F32, isOutput=False)
    w2_in = nc.declare_dram_parameter("w2", [GC_HID, GC_HID], F32, isOutput=False)
    b2_in = nc.declare_dram_parameter("b2", [1, GC_HID], F32, isOutput=False)
    # mu/ls combined: x-part [256, 64], h-part [20, 64], bias [1, 64]
    wx_in = nc.declare_dram_parameter("wx", [D_FEAT, 2 * Z_DIM], F32, isOutput=False)
    wh_in = nc.declare_dram_parameter("wh", [GC_HID, 2 * Z_DIM], F32, isOutput=False)
    bml_in = nc.declare_dram_parameter("bml", [1, 2 * Z_DIM], F32, isOutput=False)
    wd1_in = nc.declare_dram_parameter("wd1", [Z_DIM, DEC_HID], F32, isOutput=False)
    bd1_in = nc.declare_dram_parameter("bd1", [1, DEC_HID], F32, isOutput=False)
    wd2_in = nc.declare_dram_parameter("wd2", [DEC_HID, EXPR_DIM], F32, isOutput=False)
    bd2_in = nc.declare_dram_parameter("bd2", [1, EXPR_DIM], F32, isOutput=False)

    xs_out = nc.declare_dram_parameter("xs", [SH_PAD, EXPR_DIM], F32, isOutput=True)
    mu_out = nc.declare_dram_parameter("mu", [SH_PAD, Z_DIM], F32, isOutput=True)
    ls_out = nc.declare_dram_parameter("ls", [SH_PAD, Z_DIM], F32, isOutput=True)

    # collective bounce buffers
    h0_sh = nc.dram_tensor("h0_sh", [SHP, GC_HID], F32)
    H0 = nc.dram_tensor("H0", [TBL, GC_HID], F32)
    h1_sh = nc.dram_tensor("h1_sh", [SHP, GC_HID], F32)
    H1 = nc.dram_tensor("H1", [TBL, GC_HID], F32)

    K_max = int(max(K_t))
    col_off = np.concatenate([[0], np.cumsum(K_t)[:-1]]).astype(int)

    def bcast(dram_ap, n):
        return bass.AP(tensor=dram_ap.tensor, offset=dram_ap.offset,
                       ap=[[0, P]] + list(dram_ap.ap[1:]))

    from contextlib import ExitStack
    trace_sim = bool(os.environ.get("KERNEL_TRACE_SIM"))
    with tile.TileContext(nc, trace_sim=trace_sim) as tc, ExitStack() as stack:
        const = stack.enter_context(tc.tile_pool(name="const", bufs=1))
        io = stack.enter_context(tc.tile_pool(name="io", bufs=3))
        work = stack.enter_context(tc.tile_pool(name="work", bufs=3))
        wide_p = stack.enter_context(tc.tile_pool(name="wide", bufs=2))
        keep = stack.enter_context(tc.tile_pool(name="keep", bufs=1))
        ps = stack.enter_context(tc.tile_pool(name="ps", bufs=3, space="PSUM"))
        pst = stack.enter_context(tc.tile_pool(name="pst", bufs=2, space="PSUM"))

        ident = const.tile([P, P], F32)
        make_identity(nc, ident[:])

        # weights
        w1 = const.tile([P, 2, GC_HID], F32)
        nc.sync.dma_start(out=w1[:], in_=w1_in[:, :].rearrange("(a k) n -> k a n", k=P))
        wx = const.tile([P, 2, 2 * Z_DIM], F32)
        nc.sync.dma_start(out=wx[:], in_=wx_in[:, :].rearrange("(a k) n -> k a n", k=P))
        w2 = const.tile([GC_HID, GC_HID], F32)
        nc.sync.dma_start(out=w2[:], in_=w2_in[:, :])
        wh = const.tile([GC_HID, 2 * Z_DIM], F32)
        nc.sync.dma_start(out=wh[:], in_=wh_in[:, :])
        wd1 = const.tile([Z_DIM, DEC_HID], F32)
        nc.sync.dma_start(out=wd1[:], in_=wd1_in[:, :])
        wd2 = const.tile([DEC_HID, EXPR_DIM], F32)
        nc.sync.dma_start(out=wd2[:], in_=wd2_in[:, :])
        # replicated biases
        b1r = const.tile([P, GC_HID], F32)
        nc.gpsimd.dma_start(out=b1r[:], in_=bcast(b1_in[:, :], GC_HID))
        b2r = const.tile([P, GC_HID], F32)
        nc.gpsimd.dma_start(out=b2r[:], in_=bcast(b2_in[:, :], GC_HID))
        bmlr = const.tile([P, 2 * Z_DIM], F32)
        nc.gpsimd.dma_start(out=bmlr[:], in_=bcast(bml_in[:, :], 2 * Z_DIM))
        bd1r = const.tile([P, DEC_HID], F32)
        nc.gpsimd.dma_start(out=bd1r[:], in_=bcast(bd1_in[:, :], DEC_HID))
        bd2r = const.tile([P, EXPR_DIM], F32)
        nc.gpsimd.dma_start(out=bd2r[:], in_=bcast(bd2_in[:, :], EXPR_DIM))
        # per-node scalars
        rs_all = const.tile([P, TILES], F32)
        nc.sync.dma_start(out=rs_all[:], in_=rs_in[:, 0].rearrange("(t p) -> p t", p=P))
        rr_all = const.tile([P, TILES], F32)
        nc.sync.dma_start(out=rr_all[:], in_=rr_in[:, 0].rearrange("(t p) -> p t", p=P))
        # gather indices
        idxt = const.tile([P, NI], I32)
        nc.sync.dma_start(out=idxt[:], in_=idx_in[:, :])
        # persistent x-part of mu/ls
        mlx = keep.tile([P, TILES, 2 * Z_DIM], F32)

        zeros20 = const.tile([ZPAD, GC_HID], F32)
        nc.vector.memset(zeros20[:], 0.0)

        def softmax_relu_scale(hp, bias_rep, scale_col, out_tile):
            """out = softmax(relu(hp + bias), axis=-1) * scale_col.

            hp: PSUM [P, GC_HID]; bias_rep: [P, GC_HID]; scale_col: [P, 1].
            """
            hs = work.tile([P, GC_HID], F32, tag="hs")
            nc.vector.tensor_tensor(out=hs[:], in0=hp[:], in1=bias_rep[:],
                                    op=mybir.AluOpType.add)
            nc.vector.tensor_scalar_max(hs[:], hs[:], 0.0)
            # relu output is bounded (inputs ~N(0,1)); exp without max-shift is
            # safe in fp32 and mathematically identical after normalization
            ex = work.tile([P, GC_HID], F32, tag="ex")
            nc.scalar.activation(out=ex[:], in_=hs[:],
                                 func=mybir.ActivationFunctionType.Exp,
                                 bias=0.0, scale=1.0)
            sm = work.tile([P, 1], F32, tag="sm")
            nc.vector.tensor_reduce(out=sm[:], in_=ex[:], axis=mybir.AxisListType.X,
                                    op=mybir.AluOpType.add)
            rcp = work.tile([P, 1], F32, tag="rcp")
            nc.vector.reciprocal(rcp[:], sm[:])
            nc.vector.tensor_scalar(out=out_tile[:], in0=ex[:], scalar1=rcp[:, 0:1],
                                    scalar2=scale_col, op0=mybir.AluOpType.mult,
                                    op1=mybir.AluOpType.mult)

        KW = min(K_max, 256)

        def aggregate(t, table, out_tile):
            """out_tile[p,:] = sum over slots of table[idx] ; scaled by rr."""
            kt = int(K_t[t])
            if kt == 0:
                nc.vector.memset(out_tile[:], 0.0)
                return
            for c0 in range(0, kt, KW):
                cn = min(KW, kt - c0)
                wide = wide_p.tile([P, KW, GC_HID], F32, tag="wide")
                for j in range(cn):
                    col = col_off[t] + c0 + j
                    nc.gpsimd.indirect_dma_start(
                        out=wide[:, j, :], out_offset=None, in_=table[:, :],
                        in_offset=bass.IndirectOffsetOnAxis(
                            ap=idxt[:, col:col + 1], axis=0))
                if c0 == 0:
                    nc.vector.tensor_reduce(
                        out=out_tile[:], in_=wide[:, :cn, :].rearrange("p k f -> p f k"),
                        axis=mybir.AxisListType.X, op=mybir.AluOpType.add)
                else:
                    part = work.tile([P, GC_HID], F32, tag="aggpart")
                    nc.vector.tensor_reduce(
                        out=part[:], in_=wide[:, :cn, :].rearrange("p k f -> p f k"),
                        axis=mybir.AxisListType.X, op=mybir.AluOpType.add)
                    nc.vector.tensor_tensor(out=out_tile[:], in0=out_tile[:],
                                            in1=part[:], op=mybir.AluOpType.add)
            nc.vector.tensor_scalar_mul(out_tile[:], out_tile[:], rr_all[:, t:t + 1])

        def pe_transpose(src_ap, rows, cols, tag):
            """[rows<=128, cols<=128] SBUF -> [cols, rows] SBUF via PE."""
            pt = pst.tile([cols, P], F32, space="PSUM", tag="ptrans")
            nc.tensor.transpose(out=pt[:, :rows], in_=src_ap, identity=ident[:, :rows])
            st = work.tile([cols, P], F32, tag=tag)
            nc.vector.tensor_copy(out=st[:, :rows], in_=pt[:, :rows])
            return st

        # ---------------- phase 1: h0 = smax(relu(xW1+b1))*rs ; mlx = x@Wx ---
        for t in range(TILES):
            xt = io.tile([P, D_FEAT], F32, tag="xt")
            nc.sync.dma_start(out=xt[:], in_=x_in[t * P:(t + 1) * P, :])
            xT = work.tile([P, 2, P], F32, tag="xT")
            for a in range(2):
                pt = pst.tile([P, P], F32, space="PSUM", tag="ptrans")
                nc.tensor.transpose(out=pt[:], in_=xt[:, a * P:(a + 1) * P],
                                    identity=ident[:])
                nc.scalar.copy(out=xT[:, a, :], in_=pt[:])
            h0p = ps.tile([P, GC_HID], F32, space="PSUM", tag="mm")
            for a in range(2):
                nc.tensor.matmul(out=h0p[:], lhsT=xT[:, a, :], rhs=w1[:, a, :],
                                 start=(a == 0), stop=(a == 1))
            mlp = ps.tile([P, 2 * Z_DIM], F32, space="PSUM", tag="mm")
            for a in range(2):
                nc.tensor.matmul(out=mlp[:], lhsT=xT[:, a, :], rhs=wx[:, a, :],
                                 start=(a == 0), stop=(a == 1))
            nc.scalar.copy(out=mlx[:, t, :], in_=mlp[:])
            h0s = work.tile([P, GC_HID], F32, tag="hout")
            softmax_relu_scale(h0p, b1r, rs_all[:, t:t + 1], h0s)
            nc.sync.dma_start(out=h0_sh[t * P:(t + 1) * P, :], in_=h0s[:])
        nc.sync.dma_start(out=h0_sh[SH_PAD:SHP, :], in_=zeros20[:])

        # ---------------- phase 2: allgather --------------------------------
        nc.gpsimd.collective_compute(
            "AllGather", mybir.AluOpType.bypass,
            replica_groups=[list(range(NC))],
            ins=[h0_sh[:, :]], outs=[H0[:, :]],
        )

        # ---------------- phase 3: aggregate; gc2; publish h1 ----------------
        for t in range(TILES):
            m1 = work.tile([P, GC_HID], F32, tag="m1")
            aggregate(t, H0, m1)
            m1T = pe_transpose(m1[:], P, GC_HID, tag="m1T")
            h1p = ps.tile([P, GC_HID], F32, space="PSUM", tag="mm")
            nc.tensor.matmul(out=h1p[:], lhsT=m1T[:, :], rhs=w2[:, :],
                             start=True, stop=True)
            h1s = work.tile([P, GC_HID], F32, tag="hout")
            softmax_relu_scale(h1p, b2r, rs_all[:, t:t + 1], h1s)
            nc.sync.dma_start(out=h1_sh[t * P:(t + 1) * P, :], in_=h1s[:])
        nc.sync.dma_start(out=h1_sh[SH_PAD:SHP, :], in_=zeros20[:])

        # ---------------- phase 4: allgather --------------------------------
        nc.gpsimd.collective_compute(
            "AllGather", mybir.AluOpType.bypass,
            replica_groups=[list(range(NC))],
            ins=[h1_sh[:, :]], outs=[H1[:, :]],
        )

        # ---------------- phase 5+6: aggregate; heads ------------------------
        for t in range(TILES):
            h2 = work.tile([P, GC_HID], F32, tag="m1")
            aggregate(t, H1, h2)
            h2T = pe_transpose(h2[:], P, GC_HID, tag="m1T")
            mlp2 = ps.tile([P, 2 * Z_DIM], F32, space="PSUM", tag="mm")
            nc.tensor.matmul(out=mlp2[:], lhsT=h2T[:, :], rhs=wh[:, :],
                             start=True, stop=True)
            muls = work.tile([P, 2 * Z_DIM], F32, tag="muls")
            nc.vector.tensor_tensor(out=muls[:], in0=mlp2[:], in1=mlx[:, t, :],
                                    op=mybir.AluOpType.add)
            nc.vector.tensor_tensor(out=muls[:], in0=muls[:], in1=bmlr[:],
                                    op=mybir.AluOpType.add)
            nc.sync.dma_start(out=mu_out[t * P:(t + 1) * P, :], in_=muls[:, :Z_DIM])
            nc.sync.dma_start(out=ls_out[t * P:(t + 1) * P, :], in_=muls[:, Z_DIM:])
            # z = mu + (SIGMA_BOUND + exp(0.5 ls)) * eps
            sig = work.tile([P, Z_DIM], F32, tag="sig")
            nc.scalar.activation(out=sig[:], in_=muls[:, Z_DIM:],
                                 func=mybir.ActivationFunctionType.Exp,
                                 bias=0.0, scale=0.5)
            nc.vector.tensor_scalar_add(sig[:], sig[:], SIGMA_BOUND)
            epst = io.tile([P, Z_DIM], F32, tag="epst")
            nc.sync.dma_start(out=epst[:], in_=eps_in[t * P:(t + 1) * P, :])
            z = work.tile([P, Z_DIM], F32, tag="z")
            nc.vector.tensor_tensor(out=z[:], in0=sig[:], in1=epst[:],
                                    op=mybir.AluOpType.mult)
            nc.vector.tensor_tensor(out=z[:], in0=z[:], in1=muls[:, :Z_DIM],
                                    op=mybir.AluOpType.add)
            zT = pe_transpose(z[:], P, Z_DIM, tag="zT")
            dp = ps.tile([P, DEC_HID], F32, space="PSUM", tag="mm")
            nc.tensor.matmul(out=dp[:], lhsT=zT[:, :], rhs=wd1[:, :],
                             start=True, stop=True)
            d = work.tile([P, DEC_HID], F32, tag="d")
            nc.vector.tensor_tensor(out=d[:], in0=dp[:], in1=bd1r[:],
                                    op=mybir.AluOpType.add)
            nc.vector.tensor_scalar_max(d[:], d[:], 0.0)
            dT = pe_transpose(d[:], P, DEC_HID, tag="dT")
            xp = ps.tile([P, EXPR_DIM], F32, space="PSUM", tag="mm")
            nc.tensor.matmul(out=xp[:], lhsT=dT[:, :], rhs=wd2[:, :],
                             start=True, stop=True)
            xo = work.tile([P, EXPR_DIM], F32, tag="xo")
            nc.vector.tensor_tensor(out=xo[:], in0=xp[:], in1=bd2r[:],
                                    op=mybir.AluOpType.add)
            nc.sync.dma_start(out=xs_out[t * P:(t + 1) * P, :], in_=xo[:])

    nc.finalize()
    _PROGRAM_CACHE[key] = nc
    return nc


# ---------------------------------------------------------------------------
# entry point
# ---------------------------------------------------------------------------
def kernel(nodes, senders, receivers, eps,
           W_gc1, b_gc1, W_gc2, b_gc2,
           W_mu, b_mu, W_ls, b_ls,
           W_d1, b_d1, W_d2, b_d2):
    nodes = np.asarray(nodes, np.float32)
    senders = np.asarray(senders, np.int64)
    receivers = np.asarray(receivers, np.int64)
    eps = np.asarray(eps, np.float32)

    pre = _preprocess(senders, receivers)
    nc = _build_program(pre["K_t"], pre["NI"])

    # split W_mu/W_ls into h-part (first GC_HID rows) and x-part
    W_mu = np.asarray(W_mu, np.float32)
    W_ls = np.asarray(W_ls, np.float32)
    wh = np.concatenate([W_mu[:GC_HID], W_ls[:GC_HID]], axis=1)      # [20, 64]
    wx = np.concatenate([W_mu[GC_HID:], W_ls[GC_HID:]], axis=1)      # [256, 64]
    bml = np.concatenate([np.asarray(b_mu, np.float32),
                          np.asarray(b_ls, np.float32)])[None, :]    # [1, 64]

    in_maps = []
    for c in range(NC):
        ids = pre["ids"][c]
        x_c = np.zeros((SH_PAD, D_FEAT), np.float32)
        x_c[:SH_REAL] = nodes[ids]
        eps_c = np.zeros((SH_PAD, Z_DIM), np.float32)
        eps_c[:SH_REAL] = eps[ids]
        rs_c = np.zeros((SH_PAD, 1), np.float32)
        rs_c[:SH_REAL, 0] = pre["rs"][ids]
        rr_c = np.zeros((SH_PAD, 1), np.float32)
        rr_c[:SH_REAL, 0] = pre["rr"][ids]
        in_maps.append({
            "x": x_c, "eps": eps_c, "rs": rs_c, "rr": rr_c,
            "idx": pre["idx"][c],
            "w1": np.asarray(W_gc1, np.float32),
            "b1": np.asarray(b_gc1, np.float32)[None, :],
            "w2": np.asarray(W_gc2, np.float32),
            "b2": np.asarray(b_gc2, np.float32)[None, :],
            "wx": wx, "wh": wh, "bml": bml,
            "wd1": np.asarray(W_d1, np.float32),
            "bd1": np.asarray(b_d1, np.float32)[None, :],
            "wd2": np.asarray(W_d2, np.float32),
            "bd2": np.asarray(b_d2, np.float32)[None, :],
        })

    res = bass_utils.run_bass_kernel_spmd(nc, in_maps, core_ids=list(range(NC)))

    Xs = np.empty((N_NODES, EXPR_DIM), np.float32)
    mu = np.empty((N_NODES, Z_DIM), np.float32)
    ls = np.empty((N_NODES, Z_DIM), np.float32)
    for c in range(NC):
        ids = pre["ids"][c]
        r = res.results[c]
        Xs[ids] = r["xs"][:SH_REAL]
        mu[ids] = r["mu"][:SH_REAL]
        ls[ids] = r["ls"][:SH_REAL]
    return Xs, mu, ls
